# revision 1
# baseline (speedup 1.0000x reference)
"""Trainium2 Bass kernel for nn_Attention_59459527246343.

Wall-time here is dominated by the axon tunnel (measured ~60-90 MB/s
H2D, ~44 MB/s D2H, ~0.2 s fixed per launch) and per-call jit re-tracing,
not device compute (~1 ms).  Design:

  * ONE launch on 4 cores, one batch per core; the whole per-batch
    pipeline runs on-device so nothing round-trips.
  * x ships as int8 with a per-(batch,channel) absmax scale (33.5 MB);
    the device folds amax/127 into the channel preprocessing
    xt = x*(s*amax/127) + s, producing fp16 directly.
  * device: fused 1x1-conv + depthwise-3x3 as 9 shifted fp16 matmuls;
    q,k produced chunk-transposed into persistent PSUM Gram
    accumulators; v kept entirely in SBUF ([C, 65536] fp16 =
    128KB/partition); epilogue derives the l2norm row/col scales from
    the Gram diagonals, does the masked per-head softmax (off-block
    -30), forms MT = (W_proj @ A)^T, streams out = MT^T @ v.
  * out ships as int8 with a per-(row, 512-col-chunk) absmax scale
    (33.5 MB + 256 KB scales); host dequantizes to fp32.
  * a persistent jitted shard_map executor (built once, cached) avoids
    run_bass_kernel_spmd's per-call re-trace/re-compile (~2 s); output
    donation buffers are created on-device (jnp.zeros) instead of being
    pushed through the tunnel.  Falls back to run_bass_kernel_spmd if
    the fast path is unavailable.
  * quantization scratch buffers are reused across calls, and the output
    fetch streams shard-by-shard (copy_to_host_async) so each batch's
    dequant overlaps the remaining D2H.

End-to-end quantization error (numpy-simulated == hardware): ~1.0e-2
mean rel vs the 2e-2 gate.  Traffic: ~38 MB in + ~34 MB out in one
launch vs ~940 MB across two launches in the original baseline.
"""

import numpy as np
from contextlib import ExitStack

import concourse.bass as bass
from concourse.bacc import Bacc
from concourse import mybir
from concourse.tile import TileContext
from concourse.bass_utils import run_bass_kernel_spmd


def _make_runner(nc, n_cores):
    """Persistent jitted executor for `nc` on jax.devices()[:n_cores].

    run_bass_kernel_spmd builds a fresh jit closure per call, so every call
    re-traces and re-compiles the XLA wrapper (~2s).  This builds the same
    shard_map program once and reuses it; device code (NEFF) is identical.
    Returns None if the fast path is unavailable (caller falls back to
    run_bass_kernel_spmd).
    """
    try:
        import jax
        import jax.numpy as jnp
        from jax.sharding import Mesh, PartitionSpec, NamedSharding
        from jax.experimental.shard_map import shard_map
        from concourse.bass2jax import (
            _bass_exec_p, install_neuronx_cc_hook, partition_id_tensor)

        install_neuronx_cc_hook()
        partition_name = (nc.partition_id_tensor.name
                          if nc.partition_id_tensor else None)
        in_names, out_names, out_avals, out_shapes = [], [], [], []
        for alloc in nc.m.functions[0].allocations:
            if not isinstance(alloc, mybir.MemoryLocationSet):
                continue
            name = alloc.memorylocations[0].name
            if alloc.kind == "ExternalInput":
                if name != partition_name:
                    in_names.append(name)
            elif alloc.kind == "ExternalOutput":
                out_names.append(name)
                shape = tuple(alloc.tensor_shape)
                dtype = mybir.dt.np(alloc.dtype)
                out_avals.append(jax.core.ShapedArray(shape, dtype))
                out_shapes.append((shape, dtype))
        n_params = len(in_names)
        n_outs = len(out_avals)
        all_names = list(in_names) + list(out_names)
        if partition_name is not None:
            all_names.append(partition_name)
        donate = tuple(range(n_params, n_params + n_outs))

        def _body(*args):
            operands = list(args)
            if partition_name is not None:
                operands.append(partition_id_tensor())
            outs = _bass_exec_p.bind(
                *operands, out_avals=tuple(out_avals),
                in_names=tuple(all_names), out_names=tuple(out_names),
                lowering_input_output_aliases=(),
                sim_require_finite=True, sim_require_nnan=True, nc=nc)
            return tuple(outs)

        devices = jax.devices()[:n_cores]
        if len(devices) < n_cores:
            return None
        mesh = Mesh(np.asarray(devices), ("core",))
        sharded = jax.jit(
            shard_map(_body, mesh=mesh,
                      in_specs=(PartitionSpec("core"),) * (n_params + n_outs),
                      out_specs=(PartitionSpec("core"),) * n_outs,
                      check_rep=False),
            donate_argnums=donate, keep_unused=True)
        # Donation buffers filled on-device (their contents are fully
        # overwritten by the kernel) — avoids pushing zeros over the tunnel.
        shard = NamedSharding(mesh, PartitionSpec("core"))
        zero_maker = jax.jit(
            lambda: tuple(jnp.zeros((n_cores * sh[0], *sh[1:]), dt)
                          for sh, dt in out_shapes),
            out_shardings=tuple(shard for _ in out_shapes))
        # pre-create donation buffers so their on-device fill isn't
        # serialized inside the timed call; replenished off-critical-path
        zpool = [zero_maker() for _ in range(3)]

        def run(global_map, raw=False):
            # global_map: name -> [n_cores*shape0, ...] array (axis-0 concat
            # of the per-core inputs).  raw=True returns the un-fetched
            # device arrays so the caller can overlap D2H with dequant.
            import os, time
            prof = os.environ.get("KPROF")
            t0 = time.time()
            concat_in = [np.ascontiguousarray(global_map[nm])
                         for nm in in_names]
            concat_zeros = zpool.pop() if zpool else zero_maker()
            t1 = time.time()
            out_arrs = sharded(*concat_in, *concat_zeros)
            t2 = time.time()
            jax.block_until_ready(out_arrs)
            t3 = time.time()
            zpool.append(zero_maker())    # replenish off the critical path
            if raw:
                if prof:
                    print(f"[kprof] prep={t1-t0:.3f} dispatch={t2-t1:.3f} "
                          f"exec={t3-t2:.3f} (raw)", flush=True)
                return dict(zip(out_names, out_arrs))
            if os.environ.get("KNOFETCH"):
                _CACHE["out_arrs"] = out_arrs
                res = [np.zeros((n_cores * sh[0], *sh[1:]), dt)
                       for sh, dt in out_shapes]
            else:
                res = [np.asarray(a) for a in out_arrs]
            t4 = time.time()
            if prof:
                print(f"[kprof] prep={t1-t0:.3f} dispatch={t2-t1:.3f} "
                      f"exec={t3-t2:.3f} fetch={t4-t3:.3f}", flush=True)
            if os.environ.get("KSTASH"):
                _CACHE["out_arrs"] = out_arrs
            return [
                {name: res[i].reshape(n_cores, *out_shapes[i][0])[c]
                 for i, name in enumerate(out_names)}
                for c in range(n_cores)]

        return run
    except Exception:
        return None

B, C, H, W = 4, 128, 256, 256
HEADS, CH = 8, 16
N = H * W              # positions per batch
WP = W + 2             # padded row stride (zero cols at 0 and W+1)
RPT = 8                # rows per x-tile
NT = H // RPT          # 32 x-tiles
TS = 512               # phase-C chunk
NCH = N // TS          # 128 output chunks (one int8 scale per row per chunk)
F32 = mybir.dt.float32
F16 = mybir.dt.float16
I8 = mybir.dt.int8
MULT = mybir.AluOpType.mult
ADD = mybir.AluOpType.add
AX = mybir.AxisListType.X

_CACHE = {}


def _taps():
    return [(t // 3 - 1, t % 3 - 1) for t in range(9)]


def _build():
    nc = Bacc()
    xin = nc.dram_tensor("xin", [C, H, W], I8, kind="ExternalInput")
    w3 = nc.dram_tensor("w3", [C, 9, 3 * C], F16, kind="ExternalInput")
    # all small f32 inputs packed into one tensor: per-array transfers
    # through the axon tunnel carry a round-trip cost each
    # layout: [sb1 | sb2 | rsign | wpt | iden | bmask | moff]
    sm_d = nc.dram_tensor("smalls", [C, 3 + 4 * C], F32, kind="ExternalInput")
    # single output: N int8 values then the NCH f32 scales bitcast to bytes
    out = nc.dram_tensor("out", [C, N + 4 * NCH], I8, kind="ExternalOutput")

    with TileContext(nc) as tc, ExitStack() as ctx:
        consts = ctx.enter_context(tc.tile_pool(name="consts", bufs=1))
        vpers = ctx.enter_context(tc.tile_pool(name="vpers", bufs=1))
        xpool = ctx.enter_context(tc.tile_pool(name="xpool", bufs=3))
        gpool = ctx.enter_context(tc.tile_pool(name="gpool", bufs=4))
        sc = ctx.enter_context(tc.tile_pool(name="sc", bufs=1))
        opool = ctx.enter_context(tc.tile_pool(name="opool", bufs=4))
        pg = ctx.enter_context(tc.tile_pool(name="pg", bufs=2, space="PSUM"))
        pv = ctx.enter_context(tc.tile_pool(name="pv", bufs=1, space="PSUM"))
        pacc = ctx.enter_context(tc.tile_pool(name="pacc", bufs=1, space="PSUM"))
        pb = ctx.enter_context(tc.tile_pool(name="pb", bufs=1, space="PSUM"))
        pp = ctx.enter_context(tc.tile_pool(name="pp", bufs=2, space="PSUM"))

        w3_sb = consts.tile([C, 9, 3 * C], F16, tag="w3")
        nc.gpsimd.dma_start(out=w3_sb, in_=w3.ap())
        sb1_sb = consts.tile([C, 1], F32, tag="sb1")
        nc.gpsimd.dma_start(out=sb1_sb, in_=sm_d.ap()[:, 0:1])
        sb2_sb = consts.tile([C, 1], F32, tag="sb2")
        nc.gpsimd.dma_start(out=sb2_sb, in_=sm_d.ap()[:, 1:2])
        rsign_sb = consts.tile([C, 1], F32, tag="rsign")
        nc.gpsimd.dma_start(out=rsign_sb, in_=sm_d.ap()[:, 2:3])
        wpt_sb = consts.tile([C, C], F32, tag="wpt")
        nc.gpsimd.dma_start(out=wpt_sb, in_=sm_d.ap()[:, 3:3 + C])
        iden_sb = consts.tile([C, C], F32, tag="iden")
        nc.gpsimd.dma_start(out=iden_sb, in_=sm_d.ap()[:, 3 + C:3 + 2 * C])
        bmask_sb = consts.tile([C, C], F32, tag="bmask")
        nc.gpsimd.dma_start(out=bmask_sb, in_=sm_d.ap()[:, 3 + 2 * C:3 + 3 * C])
        moff_sb = consts.tile([C, C], F32, tag="moff")
        nc.gpsimd.dma_start(out=moff_sb, in_=sm_d.ap()[:, 3 + 3 * C:3 + 4 * C])
        ones1 = consts.tile([1, C], F32, tag="ones1")
        nc.vector.memset(ones1, 1.0)

        vbig = vpers.tile([C, N], F16, tag="vbig")

        gram1 = pacc.tile([C, 2 * C], F32, tag="gram1")   # [Gqq | Gqk]
        gram2 = pacc.tile([C, C], F32, tag="gram2")       # Gkk

        # dummy matmul: folds the w3-DMA dependency into PE program order so
        # real matmuls carry at most one sync-wait (ISA limit is 1).  Targets
        # the epilogue's pb bank (unused until then) to save a PSUM bank.
        dummy = pb.tile([C, C], F32, tag="pbt")
        nc.tensor.matmul(dummy, w3_sb[:, 0, 0:C], w3_sb[:, 0, 0:C],
                         start=True, stop=True)

        n_chunks = 0
        total_chunks = NT * (RPT // 2) * 4
        for it in range(NT):
            r0 = it * RPT
            lo, hi = r0 - 1, r0 + RPT + 1
            s_lo, s_hi = max(lo, 0), min(hi, H)
            ro0, ro1 = s_lo - lo, (s_lo - lo) + (s_hi - s_lo)
            xs = xpool.tile([C, RPT + 2, WP], F16, tag="xs")
            xr = xpool.tile([C, RPT + 2, W], I8, tag="xr")
            # zero pad columns (buffers rotate, so every tile) + edge rows
            nc.vector.memset(xs[:, :, 0:1], 0.0)
            nc.vector.memset(xs[:, :, WP - 1:WP], 0.0)
            if ro0 > 0:
                nc.vector.memset(xs[:, 0:ro0, :], 0.0)
            if ro1 < RPT + 2:
                nc.vector.memset(xs[:, ro1:RPT + 2, :], 0.0)
            nc.gpsimd.dma_start(out=xr[:, ro0:ro1, :],
                                in_=xin.ap()[:, s_lo:s_hi, :])
            # dequant + preprocess: xt = x_i8*(s*amax/127) + s, int8 -> fp16
            # (halo rows included; pad cols/rows stay 0)
            nc.vector.tensor_scalar(xs[:, ro0:ro1, 1:W + 1],
                                    xr[:, ro0:ro1, :],
                                    sb1_sb, sb2_sb, MULT, ADD)

            for rr in range(RPT // 2):
                # ---- v in normal orientation: psum [C, 2, W] (N=512) ----
                vps = pv.tile([C, 2, W], F32, tag="vps")
                for t9, (dy, dx) in enumerate(_taps()):
                    rhs = xs[:, 2 * rr + 1 + dy: 2 * rr + 3 + dy, 1 + dx: 1 + dx + W]
                    nc.tensor.matmul(
                        vps,
                        w3_sb[:, t9, 2 * C: 3 * C],
                        rhs,
                        start=(t9 == 0), stop=(t9 == 8),
                    )
                n0 = (r0 + 2 * rr) * W
                nc.vector.tensor_copy(vbig[:, n0:n0 + 2 * W],
                                      vps.rearrange("c a b -> c (a b)"))

                # ---- q,k transposed: 4 chunks of 128 positions ----
                for cc in range(4):
                    row = 2 * rr + cc // 2
                    wo = (cc % 2) * C
                    gps = pg.tile([C, 2 * C], F32, tag="gps")
                    for t9, (dy, dx) in enumerate(_taps()):
                        lhsT = xs[:, row + 1 + dy, 1 + dx + wo: 1 + dx + wo + C]
                        nc.tensor.matmul(
                            gps,
                            lhsT,
                            w3_sb[:, t9, 0: 2 * C],
                            start=(t9 == 0), stop=(t9 == 8),
                        )
                    gsb = gpool.tile([C, 2 * C], F16, tag="gsb")
                    nc.vector.tensor_copy(gsb, gps)
                    first = n_chunks == 0
                    last = n_chunks == total_chunks - 1
                    nc.tensor.matmul(gram1, gsb[:, 0:C],
                                     gsb, start=first, stop=last)
                    nc.tensor.matmul(gram2, gsb[:, C:2 * C],
                                     gsb[:, C:2 * C],
                                     start=first, stop=last)
                    n_chunks += 1

        # ======== epilogue: softmax + projection, all on device ========
        # Sq = diag(Gqq), Sk = diag(Gkk)
        t1 = sc.tile([C, C], F32, tag="t1")
        nc.vector.tensor_tensor(t1, gram1[:, 0:C], iden_sb, MULT)
        dq = sc.tile([C, 1], F32, tag="dq")
        nc.vector.reduce_sum(dq, t1, axis=AX)
        t2 = sc.tile([C, C], F32, tag="t2")
        nc.vector.tensor_tensor(t2, gram2, iden_sb, MULT)
        dk = sc.tile([C, 1], F32, tag="dk")
        nc.vector.reduce_sum(dk, t2, axis=AX)
        # rowscale = temp*sign(q_pre)/sqrt(Sq); colscale = 1/sqrt(Sk)
        sqq = sc.tile([C, 1], F32, tag="sqq")
        nc.scalar.sqrt(sqq, dq)
        rq = sc.tile([C, 1], F32, tag="rq")
        nc.vector.reciprocal(rq, sqq)
        rowscale = sc.tile([C, 1], F32, tag="rowscale")
        nc.vector.tensor_tensor(rowscale, rq, rsign_sb, MULT)
        sqk = sc.tile([C, 1], F32, tag="sqk")
        nc.scalar.sqrt(sqk, dk)
        rk = sc.tile([C, 1], F32, tag="rk")
        nc.vector.reciprocal(rk, sqk)
        # transpose colscale to a row, broadcast to [C, C], fold block mask
        # (pb bank is reused sequentially: transpose -> broadcast -> MT)
        tpt = pb.tile([C, C], F32, tag="pbt")
        nc.tensor.matmul(tpt[0:1, :], rk, iden_sb, start=True, stop=True)
        tsb = sc.tile([1, C], F32, tag="tsb")
        nc.vector.tensor_copy(tsb, tpt[0:1, :])
        cbp = pb.tile([C, C], F32, tag="pbt")
        nc.tensor.matmul(cbp, ones1, tsb, start=True, stop=True)
        cbm = sc.tile([C, C], F32, tag="cbm")
        nc.vector.tensor_tensor(cbm, cbp, bmask_sb, MULT)
        # L = (Gqk * rowscale) * (colscale*mask) + moff ; masked softmax
        lt = sc.tile([C, C], F32, tag="lt")
        nc.vector.scalar_tensor_tensor(lt, gram1[:, C:2 * C], rowscale, cbm,
                                       MULT, MULT)
        nc.vector.tensor_tensor(lt, lt, moff_sb, ADD)
        mx = sc.tile([C, 1], F32, tag="mx")
        nc.vector.reduce_max(mx, lt, axis=AX)
        nmx = sc.tile([C, 1], F32, tag="nmx")
        nc.vector.tensor_scalar_mul(nmx, mx, -1.0)
        ex = sc.tile([C, C], F32, tag="ex")
        rs = sc.tile([C, 1], F32, tag="rs")
        nc.scalar.activation(ex, lt, mybir.ActivationFunctionType.Exp,
                             bias=nmx, scale=1.0, accum_out=rs)
        rrec = sc.tile([C, 1], F32, tag="rrec")
        nc.vector.reciprocal(rrec, rs)
        asb = sc.tile([C, C], F32, tag="asb")
        nc.vector.tensor_scalar_mul(asb, ex, rrec)
        # MT[d, o] = sum_c A[c, d] * W_proj[o, c]  (= (W_proj @ A)^T)
        mtp = pb.tile([C, C], F32, tag="pbt")
        nc.tensor.matmul(mtp, asb, wpt_sb, start=True, stop=True)
        mt16 = sc.tile([C, C], F16, tag="mt16")
        nc.vector.tensor_copy(mt16, mtp)

        # ======== phase C: out = MT^T @ v, int8-quantized per row-chunk ====
        scsb = sc.tile([C, NCH], F32, tag="scsb")
        rmax = sc.tile([C, 1], F32, tag="rmax")
        rq = sc.tile([C, 1], F32, tag="rq")
        for i in range(NCH):
            ops = pp.tile([C, TS], F32, tag="ops")
            nc.tensor.matmul(ops, mt16, vbig[:, TS * i: TS * (i + 1)],
                             start=True, stop=True)
            nc.vector.reduce_max(rmax, ops, axis=AX, apply_absolute_value=True)
            nc.vector.tensor_scalar_max(scsb[:, i:i + 1], rmax, 1e-30)
            nc.vector.reciprocal(rq, scsb[:, i:i + 1])
            osb = opool.tile([C, TS], I8, tag="osb")
            nc.vector.tensor_scalar(osb, ops, rq, 127.0, MULT, MULT)
            nc.sync.dma_start(out=out.ap()[:, TS * i: TS * (i + 1)], in_=osb)
        nc.sync.dma_start(out=out.ap()[:, N:N + 4 * NCH].bitcast(F32),
                          in_=scsb)
    nc.compile()
    return nc


def kernel(x, p, temperature, W_qkv, W_dw, W_proj, W_kp):
    x = np.asarray(x, np.float32)
    p = np.asarray(p, np.float32)
    temperature = np.asarray(temperature, np.float32)
    W_qkv = np.asarray(W_qkv, np.float32)
    W_dw = np.asarray(W_dw, np.float32)
    W_proj = np.asarray(W_proj, np.float32)
    W_kp = np.asarray(W_kp, np.float32)

    if "k" not in _CACHE:
        _CACHE["k"] = _build()
        _CACHE["runner"] = _make_runner(_CACHE["k"], B)
    nc = _CACHE["k"]

    s = (p[:, :C] + p[:, C:]).astype(np.float32)  # [B, C]
    q_pre = p @ W_kp.T                            # [B, C]
    W_dw9 = W_dw[:, 0].reshape(3 * C, 9)          # [o, t]
    w3 = (W_qkv.T[:, None, :] * W_dw9.T[None, :, :]).astype(np.float16)
    w3 = np.ascontiguousarray(w3)                 # [C, 9, 3C] fp16

    # int8-quantize x with a per-(batch, channel) absmax scale; the device
    # folds amax/127 into the preprocessing multiply.  Scratch buffers are
    # reused across calls (they are internal; the transfer completes before
    # kernel() returns, so overwriting next call is safe).
    amax = np.maximum(np.maximum(x.max(axis=(2, 3)), -x.min(axis=(2, 3))),
                      1e-30)                                 # [B, C]
    if "qbufs" not in _CACHE:
        _CACHE["qbufs"] = (np.empty((B, C, H, W), np.float32),
                           np.empty((B, C, H, W), np.int8))
    y, xi8 = _CACHE["qbufs"]
    np.multiply(x, (127.0 / amax)[:, :, None, None], out=y)
    np.rint(y, out=y)
    np.copyto(xi8, y, casting="unsafe")       # y holds exact integers
    xi8g = xi8.reshape(B * C, H, W)
    sb1_g = (s * amax * np.float32(1.0 / 127.0)).astype(
        np.float32).reshape(B * C, 1)

    if "smbuf" not in _CACHE:
        sm = np.empty((B, C, 3 + 4 * C), np.float32)
        iden = np.eye(C, dtype=np.float32)
        bmask = np.zeros((C, C), np.float32)
        for h in range(HEADS):
            bmask[CH * h:CH * (h + 1), CH * h:CH * (h + 1)] = 1.0
        sm[:, :, 3 + C:3 + 2 * C] = iden
        sm[:, :, 3 + 2 * C:3 + 3 * C] = bmask
        sm[:, :, 3 + 3 * C:3 + 4 * C] = (bmask - 1.0) * 30.0
        _CACHE["smbuf"] = sm
    sm = _CACHE["smbuf"]
    temp_pc = np.repeat(temperature[:, 0, 0].astype(np.float32), CH)  # [C]
    sm[:, :, 0] = sb1_g.reshape(B, C)
    sm[:, :, 1] = s
    sm[:, :, 2] = temp_pc[None, :] * np.sign(q_pre)
    sm[:, :, 3:3 + C] = np.ascontiguousarray(W_proj.T)

    global_map = {
        "xin": xi8g, "w3": np.tile(w3, (B, 1, 1)),
        "smalls": sm.reshape(B * C, 3 + 4 * C),
    }

    runner = _CACHE.get("runner")
    if runner is not None:
        try:
            arrs = runner(global_map, raw=True)
            # stream the single int8 output shard by shard: async copies
            # keep the tunnel busy while each fetched batch is dequantized
            # (each row carries its NCH f32 scales bitcast into the tail)
            out = np.empty((B, C, H, W), np.float32)
            shards = list(arrs["out"].addressable_shards)
            for sh in shards:
                sh.data.copy_to_host_async()
            for sh in shards:
                st = sh.index[0].start
                b = 0 if st is None else st // C
                raw = np.asarray(sh.data)                  # [C, N + 4*NCH]
                i8 = raw[:, :N].reshape(C, NCH, TS)
                scale = raw[:, N:].view(np.float32) * np.float32(1.0 / 127.0)
                np.multiply(i8, scale[:, :, None],
                            out=out[b].reshape(C, NCH, TS), casting="unsafe")
            return out
        except Exception:
            _CACHE["runner"] = None

    in_maps = [
        {nm: ar.reshape(B, ar.shape[0] // B, *ar.shape[1:])[b]
         for nm, ar in global_map.items()}
        for b in range(B)]
    results = run_bass_kernel_spmd(
        nc, in_maps, core_ids=list(range(B))).results
    out = np.empty((B, C, H, W), np.float32)
    for b in range(B):
        raw = results[b]["out"]                            # [C, N + 4*NCH]
        i8 = raw[:, :N].reshape(C, NCH, TS)
        scale = raw[:, N:].view(np.float32) * np.float32(1.0 / 127.0)
        np.multiply(i8, scale[:, :, None],
                    out=out[b].reshape(C, NCH, TS), casting="unsafe")
    return out

